# revision 11
# baseline (speedup 1.0000x reference)
"""AttnAggregator2 Trainium2 kernel.

Math (per node n, with X[n, s, :] = table rows of [self, neigh_0..neigh_24]):
    Q       = table[node] @ Wq^T + bq
    scores  = Q . K  where K = X @ Wk^T + bk
            = (Q @ Wk) . X + (Q . bk)          <- Q.bk is constant per node and
                                                  cancels in softmax: dropped.
    attn    = softmax(scores)
    mix     = attn-weighted sum of V = (sum_s attn_s X_s) @ Wv^T + bv
                                                  (sum attn = 1 absorbs bv)

So the S+1 per-neighbor K/V projections collapse into three small dense
matmuls per node tile plus one elementwise product pass (scores) and one
PE "diagonal matmul" accumulation (the attn-weighted feature sum).

Sharding: data-parallel over nodes, 8 cores, table + weights replicated.

Per-core layout (node tiles of 128 on SBUF partitions):
    gather   G[p, s, :]  = table[idx[p, s]]           (indirect DMA, fp32)
    Q^T      = Wq @ Xself^T        (PE; Xself^T via PE transpose)
    Q'       = Q @ Wk              (PE, row layout [n, d])
    prod     = G * broadcast_s(Q')                    (DVE, fp32)
    scores   = reduce_d(prod)                         (DVE, fp32)
    attn     = softmax over s                         (DVE + ACT)
    diag_s   = diag(attn[:, s])   (DVE: bf16 identity x broadcast attn)
    Xmix^T   = sum_s (G_s)^T @ diag_s                 (PE, bf16, PSUM accum)
    out^T    = Wv @ Xmix^T + bv                       (PE fp32)
Output is written transposed [128, n]; host transposes back.
"""

import os
import sys
from contextlib import ExitStack

import numpy as np

sys.path.insert(0, "/opt/trn_rl_repo")

import concourse.bass as bass
import concourse.mybir as mybir
import concourse.tile as tile
from concourse import bacc
from concourse.bass_utils import run_bass_kernel_spmd
from concourse.masks import make_identity

F32 = mybir.dt.float32
BF16 = mybir.dt.bfloat16
I32 = mybir.dt.int32

VOCAB = 100000
N_NODES = 50000
S = 25
S1 = S + 1  # self + sampled neighbors
D = 128
P = 128
N_CORES = 8
N_PER_CORE = N_NODES // N_CORES  # 6250
N_TILES = (N_PER_CORE + P - 1) // P  # 49
N_PAD = N_TILES * P  # 6272


def build_kernel(n_tiles: int = N_TILES, vocab: int = VOCAB):
    nc = bacc.Bacc(
        "TRN2",
        target_bir_lowering=False,
        debug=False,
        enable_asserts=False,
    )

    table = nc.dram_tensor("table", [vocab, D], F32, kind="ExternalInput").ap()
    idx = nc.dram_tensor("idx", [P, n_tiles * S1], I32, kind="ExternalInput").ap()
    wqT = nc.dram_tensor("wqT", [D, D], F32, kind="ExternalInput").ap()
    wk = nc.dram_tensor("wk", [D, D], F32, kind="ExternalInput").ap()
    wvT = nc.dram_tensor("wvT", [D, D], F32, kind="ExternalInput").ap()
    bq = nc.dram_tensor("bq", [D, 1], F32, kind="ExternalInput").ap()
    bv = nc.dram_tensor("bv", [D, 1], F32, kind="ExternalInput").ap()
    out = nc.dram_tensor("out", [D, n_tiles * P], F32, kind="ExternalOutput").ap()

    with tile.TileContext(nc) as tc, ExitStack() as ctx:
        const = ctx.enter_context(tc.tile_pool(name="const", bufs=1))
        idxp = ctx.enter_context(tc.tile_pool(name="idxp", bufs=3))
        gpool = ctx.enter_context(tc.tile_pool(name="gpool", bufs=3))
        gbfp = ctx.enter_context(tc.tile_pool(name="gbfp", bufs=2))
        prodp = ctx.enter_context(tc.tile_pool(name="prodp", bufs=2))
        diagp = ctx.enter_context(tc.tile_pool(name="diagp", bufs=2))
        small = ctx.enter_context(tc.tile_pool(name="small", bufs=4))
        outp = ctx.enter_context(tc.tile_pool(name="outp", bufs=3))
        psum = ctx.enter_context(tc.tile_pool(name="psum", bufs=1, space="PSUM"))
        psum_xm = ctx.enter_context(tc.tile_pool(name="psum_xm", bufs=2, space="PSUM"))

        ident = const.tile([P, P], F32)
        make_identity(nc, ident[:])
        ident_bf = const.tile([P, P], BF16)
        nc.scalar.copy(ident_bf[:], ident[:])
        wqT_s = const.tile([D, D], F32)
        nc.sync.dma_start(wqT_s[:], wqT)
        wk_s = const.tile([D, D], F32)
        nc.sync.dma_start(wk_s[:], wk)
        wvT_s = const.tile([D, D], F32)
        nc.sync.dma_start(wvT_s[:], wvT)
        bq_s = const.tile([D, 1], F32)
        nc.sync.dma_start(bq_s[:], bq)
        bv_s = const.tile([D, 1], F32)
        nc.sync.dma_start(bv_s[:], bv)
        idx_all = const.tile([P, n_tiles * S1], I32)
        nc.sync.dma_start(idx_all[:], idx)

        for t in range(n_tiles):
            # Gather all S1 rows for 128 nodes: G[p, s, :] = table[idx[p, s]]
            # (one indirect DMA per s-slot: HW only supports one offset per
            # partition per call)
            g = gpool.tile([P, S1, D], F32)
            for s in range(S1):
                nc.gpsimd.indirect_dma_start(
                    out=g[:, s, :],
                    out_offset=None,
                    in_=table,
                    in_offset=bass.IndirectOffsetOnAxis(
                        ap=idx_all[:, t * S1 + s : t * S1 + s + 1], axis=0
                    ),
                    oob_is_err=False,
                )

            # Xself^T via PE transpose
            ps_xsT = psum.tile([P, P], F32)
            nc.tensor.transpose(ps_xsT[:], g[:, 0, :], ident[:])
            xsT = small.tile([P, P], F32)
            nc.scalar.copy(xsT[:], ps_xsT[:])

            # Q^T = Wq @ Xself^T + bq   [j, n]
            ps_qT = psum.tile([P, P], F32)
            nc.tensor.matmul(ps_qT[:], lhsT=wqT_s[:], rhs=xsT[:], start=True, stop=True)
            qT = small.tile([P, P], F32)
            nc.scalar.activation(
                qT[:],
                ps_qT[:],
                func=mybir.ActivationFunctionType.Identity,
                bias=bq_s[:, :1],
            )

            # Q' = Q @ Wk   [n, d]  (lhsT = Q^T)
            ps_qp = psum.tile([P, P], F32)
            nc.tensor.matmul(ps_qp[:], lhsT=qT[:], rhs=wk_s[:], start=True, stop=True)
            qp = small.tile([P, P], F32)
            nc.scalar.copy(qp[:], ps_qp[:])

            # scores_s[n] = sum_d G[n, s, d] * Q'[n, d]
            prod = prodp.tile([P, S1, D], F32)
            nc.vector.tensor_tensor(
                prod[:],
                g[:],
                qp[:, None, :].to_broadcast([P, S1, D]),
                op=mybir.AluOpType.mult,
            )
            sc = small.tile([P, S1], F32)
            nc.vector.tensor_reduce(
                sc[:], prod[:], axis=mybir.AxisListType.X, op=mybir.AluOpType.add
            )

            # softmax over s (free dim)
            negmax = small.tile([P, 1], F32)
            nc.vector.tensor_reduce(
                negmax[:],
                sc[:],
                axis=mybir.AxisListType.X,
                op=mybir.AluOpType.max,
                negate=True,
            )
            e = small.tile([P, S1], F32)
            zsum = small.tile([P, 1], F32)
            nc.scalar.activation(
                e[:],
                sc[:],
                func=mybir.ActivationFunctionType.Exp,
                bias=negmax[:, :1],
                accum_out=zsum[:],
            )
            zinv = small.tile([P, 1], F32)
            nc.vector.reciprocal(zinv[:], zsum[:])
            attn = small.tile([P, S1], BF16)
            nc.vector.tensor_scalar_mul(attn[:], e[:], zinv[:, :1])

            # diag_all[p, s, y] = attn[p, s] if p == y else 0  (DVE — gpsimd is
            # saturated by gather descriptor generation)
            diag = diagp.tile([P, S1, D], BF16)
            nc.vector.tensor_tensor(
                diag[:],
                ident_bf[:, None, :].to_broadcast([P, S1, D]),
                attn[:, :, None].to_broadcast([P, S1, D]),
                op=mybir.AluOpType.mult,
            )

            # bf16 copy of gathered rows for the PE weighted-sum
            gbf = gbfp.tile([P, S1, D], BF16)
            nc.scalar.copy(gbf[:], g[:])

            # Xmix^T = sum_s (G_s)^T @ diag(attn_s)   [d, n]
            ps_xm = psum_xm.tile([P, P], F32)
            for s in range(S1):
                nc.tensor.matmul(
                    ps_xm[:],
                    lhsT=gbf[:, s, :],
                    rhs=diag[:, s, :],
                    start=(s == 0),
                    stop=(s == S1 - 1),
                )
            xmT = small.tile([P, P], F32)
            nc.scalar.copy(xmT[:], ps_xm[:])

            # out^T = Wv @ Xmix^T + bv   [j, n]
            ps_mx = psum.tile([P, P], F32)
            nc.tensor.matmul(ps_mx[:], lhsT=wvT_s[:], rhs=xmT[:], start=True, stop=True)
            o_t = outp.tile([P, P], F32)
            nc.scalar.activation(
                o_t[:],
                ps_mx[:],
                func=mybir.ActivationFunctionType.Identity,
                bias=bv_s[:, :1],
            )
            nc.sync.dma_start(out[:, bass.ts(t, P)], o_t[:])

    nc.compile()
    return nc


_NC_CACHE = {}


def _get_nc():
    key = (N_TILES, VOCAB)
    if key not in _NC_CACHE:
        _NC_CACHE[key] = build_kernel()
    return _NC_CACHE[key]


def kernel(**inputs) -> np.ndarray:
    table = np.ascontiguousarray(np.asarray(inputs["table"], dtype=np.float32))
    node = np.asarray(inputs["node"]).astype(np.int32)
    neighs = np.asarray(inputs["neighs"]).astype(np.int32)
    Wq = np.asarray(inputs["Wq"], dtype=np.float32)
    bq = np.asarray(inputs["bq"], dtype=np.float32)
    Wk = np.asarray(inputs["Wk"], dtype=np.float32)
    Wv = np.asarray(inputs["Wv"], dtype=np.float32)
    bv = np.asarray(inputs["bv"], dtype=np.float32)

    idx_full = np.concatenate([node[:, None], neighs], axis=1)  # [N, S1] int32

    common = {
        "table": table,
        "wqT": np.ascontiguousarray(Wq.T),
        "wk": np.ascontiguousarray(Wk),
        "wvT": np.ascontiguousarray(Wv.T),
        "bq": np.ascontiguousarray(bq[:, None]),
        "bv": np.ascontiguousarray(bv[:, None]),
    }

    in_maps = []
    for c in range(N_CORES):
        idx_c = idx_full[c * N_PER_CORE : (c + 1) * N_PER_CORE]
        idx_pad = np.zeros((N_PAD, S1), dtype=np.int32)
        idx_pad[:N_PER_CORE] = idx_c
        in_maps.append(dict(common, idx=np.ascontiguousarray(
            idx_pad.reshape(N_TILES, P, S1).transpose(1, 0, 2).reshape(P, N_TILES * S1)
        )))

    nc = _get_nc()
    results = run_bass_kernel_spmd(nc, in_maps, list(range(N_CORES))).results

    out = np.empty((N_NODES, D), dtype=np.float32)
    for c in range(N_CORES):
        out[c * N_PER_CORE : (c + 1) * N_PER_CORE] = results[c]["out"][
            :, :N_PER_CORE
        ].T
    return out


if __name__ == "__main__":
    rng = np.random.default_rng(0)
    inputs = {
        "table": rng.standard_normal((VOCAB, D), dtype=np.float32),
        "node": rng.integers(0, VOCAB, (N_NODES,)),
        "neighs": rng.integers(0, VOCAB, (N_NODES, S)),
        "Wq": rng.uniform(-0.09, 0.09, (D, D)).astype(np.float32),
        "bq": rng.uniform(-0.09, 0.09, (D,)).astype(np.float32),
        "Wk": rng.uniform(-0.09, 0.09, (D, D)).astype(np.float32),
        "bk": rng.uniform(-0.09, 0.09, (D,)).astype(np.float32),
        "Wv": rng.uniform(-0.09, 0.09, (D, D)).astype(np.float32),
        "bv": rng.uniform(-0.09, 0.09, (D,)).astype(np.float32),
    }
    res = kernel(**inputs)
    print("kernel ran, output shape", res.shape)
